# revision 22
# baseline (speedup 1.0000x reference)
"""CliffordAttention TRN2 kernel.

Math: the reference's orientation-bias einsum chain is folded into the Wq
projection. With A_h[i,j] = scale*rev[j]*met[j]*delta_ij + rev[i]*wk[h,i,j]
(wk = biv_kernel . bias_weight), the total pre-softmax logits are exactly
q_eff . k where q_eff = x @ Wq_eff.T and Wq_eff mixes blades of Wq by A_h.
bk shifts every logit in a softmax row equally (drops out exactly); bv adds
bv to the output exactly (softmax rows sum to 1) and is applied on host.

Sharding: tensor-parallel over heads — core c computes heads 2c, 2c+1 for
all batches. x (feature-major, bf16) is replicated; W slices per core.
Device kernel per core: q/k projections in feature-major layout (qT/kT),
v in row-major, then per (b, head): S.T = k q^T (m on partitions), P=exp(S.T)
(no max subtraction needed: |logits| < ~20), out.T = v^T P accumulated on PE,
denominator partials as a DVE running sum (host reduces over partitions,
divides, transposes, concats).

The kernel is PE-bound at ~97% of the bf16 roofline (3512 N=512 matmuls,
~777us busy). Scheduling notes that keep it there: HAM warm-up matmuls
during the initial DMA wait; just-in-time weight-DMA spreading; kt-paired
3D x DMAs; in the attention phase, ACT runs exps only, DVE does denominator
adds + evacuations, ot matmuls trail their exp by two key tiles, and output
rides bf16 over two DMA queues. Measured HW exec ~802-808us/core; runs can
show +7-20% when the chip's P0 power throttle drops the PE to ~2GHz.
"""

import os

import numpy as np
import ml_dtypes

BF16 = ml_dtypes.bfloat16

# Problem shapes (hardcoded per contract).
B = 4
L = 1024
NH = 16
CD = 32
NB = 8
DM = NH * CD * NB  # 4096
DH = CD * NB  # 256 head dim
N_CORES = 8
HPC = NH // N_CORES  # heads per core
FPC = HPC * DH  # 512 output features per core
ROWS = B * L  # 4096
SIG_G = (1.0, 1.0, 1.0)
BIV_IDX = (4, 5, 6)


def _build_tables():
    """Geometric-product structure constants; copied from the model spec."""
    g = SIG_G
    dim = len(g)
    n = 2**dim
    masks = sorted(range(n), key=lambda m: (bin(m).count("1"), m))
    inv = {m: i for i, m in enumerate(masks)}
    C = np.zeros((n, n, n), dtype=np.float32)
    for i, a in enumerate(masks):
        for j, b in enumerate(masks):
            sw, at = 0, a >> 1
            while at:
                sw += bin(at & b).count("1")
                at >>= 1
            s = -1.0 if (sw & 1) else 1.0
            for t in range(dim):
                if (a >> t) & 1 and (b >> t) & 1:
                    s *= float(g[t])
            C[i, j, inv[a ^ b]] = s
    grades = np.array([bin(m).count("1") for m in masks])
    rev = np.where((grades * (grades - 1) // 2) % 2 == 1, -1.0, 1.0).astype(np.float32)
    met = np.array(
        [
            float(np.prod([g[t] for t in range(dim) if (m >> t) & 1] + [1.0]))
            for m in masks
        ],
        dtype=np.float32,
    )
    return C, rev, met


_C_NP, _REV_NP, _MET_NP = _build_tables()

# Device-kernel tiling constants.
KT = DM // 128  # 32 contraction tiles
NFT = FPC // 128  # 4 output-feature tiles per core
N_RQ = ROWS // 1024  # 4 row quads
N_MT = L // 128  # 8 key tiles per (b, h)
N_LB = L // 512  # 2 query blocks per (b, h)
N_KF = DH // 128  # 2 feature tiles per head

_CACHE = {}


def _build_nc():
    import concourse.tile as tile
    from concourse import bacc, mybir

    f32 = mybir.dt.float32
    bf16 = mybir.dt.bfloat16
    Exp = mybir.ActivationFunctionType.Exp
    CopyF = mybir.ActivationFunctionType.Copy
    IdentF = mybir.ActivationFunctionType.Identity

    nc = bacc.Bacc(
        "TRN2",
        target_bir_lowering=False,
        debug=False,
        enable_asserts=False,
        num_devices=N_CORES,
    )

    xT_d = nc.dram_tensor("xT", [DM, ROWS], bf16, kind="ExternalInput").ap()
    wqT_d = nc.dram_tensor("wqT", [DM, FPC], bf16, kind="ExternalInput").ap()
    wkT_d = nc.dram_tensor("wkT", [DM, FPC], bf16, kind="ExternalInput").ap()
    wvT_d = nc.dram_tensor("wvT", [DM, FPC], bf16, kind="ExternalInput").ap()
    bqe_d = nc.dram_tensor("bqe", [FPC], f32, kind="ExternalInput").ap()
    outT_d = nc.dram_tensor("outT", [FPC, ROWS], bf16, kind="ExternalOutput").ap()
    # denominator partial sums: summed over the partition axis on host
    den_d = nc.dram_tensor(
        "den", [HPC * B, 128, L], f32, kind="ExternalOutput"
    ).ap()

    with tile.TileContext(nc) as tc:
        # HAM warm-up: the PE clock-gate opens only after ~3.4us of sustained
        # matmul activity. The first real matmul can't start until weights+x
        # arrive (~10us); burn the DMA-wait window on throwaway matmuls so the
        # real stream starts at 2.4GHz instead of 1.2.
        with (
            tc.tile_pool(name="warm_ps", bufs=1, space="PSUM") as wpsp,
            tc.tile_pool(name="warm_sb", bufs=1) as wsbp,
        ):
            wsrc = wsbp.tile([128, 512], bf16, tag="wsrc", name="wsrc")
            nc.vector.memset(wsrc[:], 0)
            wps = wpsp.tile([128, 512], f32, tag="wps", name="wps")
            for _ in range(8):
                nc.tensor.matmul(
                    wps[:], lhsT=wsrc[:, 0:128], rhs=wsrc[:], start=True, stop=True
                )

        with tc.tile_pool(name="persist", bufs=1) as persist:
            qT = [
                persist.tile([128, ROWS], bf16, tag=f"qT{f}", name=f"qT{f}")
                for f in range(NFT)
            ]
            kT = [
                persist.tile([128, ROWS], bf16, tag=f"kT{f}", name=f"kT{f}")
                for f in range(NFT)
            ]
            vv = persist.tile([128, ROWS // 128, FPC], bf16, tag="vv", name="vv")
            bq_sb = persist.tile([128, NFT], f32, tag="bq", name="bq_sb")

            # ---- projections ----
            # Weights live in per-k-tile tiles (32 tags, bufs=2) so the first
            # matmul of a phase only waits on one small DMA, and the next
            # phase's weights prefetch during the current phase's compute.
            with (
                tc.tile_pool(name="wpool", bufs=2) as wp,
                tc.tile_pool(name="xs", bufs=16) as xp,
                tc.tile_pool(name="psp", bufs=1, space="PSUM") as pp,
            ):
                # Weight DMAs are spread just-in-time through the rblk loop:
                # issuing all 12.6MB upfront saturates HBM across the 8 cores
                # and starves the x stream (PE stalls ~45us in once the x
                # prefetch buffer drains).
                wq4 = wqT_d.rearrange("(kt p) f -> p kt f", p=128)
                wk4 = wkT_d.rearrange("(kt p) f -> p kt f", p=128)
                wv4 = wvT_d.rearrange("(kt p) f -> p kt f", p=128)
                x3 = xT_d.rearrange("(kt p) r -> p kt r", p=128)
                wq_t, wk_t, wv_t = [], [], []

                def issue_w(kind, kt):
                    lst, src = {"q": (wq_t, wq4), "k": (wk_t, wk4), "v": (wv_t, wv4)}[kind]
                    wt = wp.tile([128, FPC], bf16, tag=f"w{kt}", name=f"w{kind}{kt}")
                    nc.scalar.dma_start(out=wt, in_=src[:, kt, :])
                    lst.append(wt)

                # NOTE: splitting rblk0 into q-only + k-only passes (to halve
                # startup weight demand) was tried and is a NET LOSS (+17us):
                # the later passes' x DMAs get pulled forward by pool-buffer
                # rotation and starve whichever pass is running. Keep fused.
                QK_PREFETCH = 8
                for kt in range(QK_PREFETCH):
                    issue_w("q", kt)
                    issue_w("k", kt)
                nc.scalar.dma_start(
                    out=bq_sb, in_=bqe_d.rearrange("(f p) -> p f", p=128)
                )

                def proj_pass(rblk, do_q, do_k, w_sched, xpool=None):
                    """One 512-row pass; w_sched: ktp -> list of (kind, kt)."""
                    xpool = xpool or xp
                    pq = [
                        pp.tile([128, 512], f32, tag=f"ps{i}", name=f"psq{i}")
                        for i in range(NFT)
                    ] if do_q else None
                    pk = [
                        pp.tile([128, 512], f32, tag=f"ps{NFT + i}", name=f"psk{i}")
                        for i in range(NFT)
                    ] if do_k else None
                    # x rides in kt-PAIRED 3D DMAs ([128, 2, 512]): half the
                    # descriptor traffic on the Sync queue and twice the
                    # prefetch runway per pool buffer
                    for ktp in range(KT // 2):
                        for kind, kt in w_sched(ktp):
                            issue_w(kind, kt)
                        xt = xpool.tile([128, 2, 512], bf16, tag="x", name="xqkt")
                        nc.sync.dma_start(
                            out=xt,
                            in_=x3[
                                :,
                                2 * ktp : 2 * ktp + 2,
                                rblk * 512 : (rblk + 1) * 512,
                            ],
                        )
                        for half in range(2):
                            kt = 2 * ktp + half
                            if do_q:
                                for f in range(NFT):
                                    nc.tensor.matmul(
                                        pq[f][:],
                                        lhsT=wq_t[kt][:, f * 128 : (f + 1) * 128],
                                        rhs=xt[:, half, :],
                                        start=(kt == 0),
                                        stop=(kt == KT - 1),
                                    )
                            if do_k:
                                for f in range(NFT):
                                    nc.tensor.matmul(
                                        pk[f][:],
                                        lhsT=wk_t[kt][:, f * 128 : (f + 1) * 128],
                                        rhs=xt[:, half, :],
                                        start=(kt == 0),
                                        stop=(kt == KT - 1),
                                    )
                    # evacuate on both Vector and Scalar so the PSUM banks
                    # free ~2x faster at block boundaries
                    if do_q:
                        for f in range(NFT):
                            dst = qT[f][:, rblk * 512 : (rblk + 1) * 512]
                            if f % 2 == 0:
                                nc.vector.tensor_scalar_add(
                                    dst, pq[f][:], bq_sb[:, f : f + 1]
                                )
                            else:
                                nc.scalar.activation(
                                    dst, pq[f][:], IdentF, bias=bq_sb[:, f : f + 1]
                                )
                    if do_k:
                        # DVE takes k0,k2,k3; ACT only k1: ACT activations
                        # cost ~0.7us vs DVE's 0.43, and the latest-stopping
                        # bank (k3) on ACT misses its reuse window at the
                        # next rblk's group start (the steady 432ns PE gaps
                        # hit exactly the ACT-evacuated psum banks)
                        for f in range(NFT):
                            dst = kT[f][:, rblk * 512 : (rblk + 1) * 512]
                            if f == 1:
                                nc.scalar.activation(dst, pk[f][:], CopyF)
                            else:
                                nc.vector.tensor_copy(dst, pk[f][:])

                def rblk0_sched(ktp):
                    if ktp >= 12:
                        return []
                    out = []
                    for kt in (QK_PREFETCH + 2 * ktp, QK_PREFETCH + 2 * ktp + 1):
                        out += [("q", kt), ("k", kt)]
                    return out

                for rblk in range(ROWS // 512):
                    # weight DMAs spread just-in-time: rblk0 trickles the
                    # remaining wq/wk, rblk1 issues wv (gated behind rblk0
                    # evacs in the scalar queue so it can't starve x)
                    if rblk == 0:
                        sched = rblk0_sched
                    elif rblk == 1:
                        sched = lambda ktp: [("v", 2 * ktp), ("v", 2 * ktp + 1)]
                    else:
                        sched = lambda ktp: []
                    proj_pass(rblk, True, True, sched)

                # v projection: out[row_tile, f] = xT.T @ wvT (row-major v).
                # Uniform 4-bank half-quad passes: evacuations of one pass
                # overlap the next pass's matmuls, and at the v->attention
                # transition half the PSUM banks are already free (otherwise
                # the first st matmul waits ~1.2us on the final v evacs).
                # x DMAs are kt-paired like the q/k phase.
                for half_q in range(2 * N_RQ):
                    rq, pas = divmod(half_q, 2)
                    grp = [4 * pas + j for j in range(4)]
                    base = rq * 1024 + pas * 512
                    pst = {
                        rt: pp.tile([128, FPC], f32, tag=f"ps{rt}", name=f"psv{rt}")
                        for rt in grp
                    }
                    for ktp in range(KT // 2):
                        xt = xp.tile([128, 2, 512], bf16, tag="x", name="xvt")
                        nc.sync.dma_start(
                            out=xt,
                            in_=x3[:, 2 * ktp : 2 * ktp + 2, base : base + 512],
                        )
                        for half in range(2):
                            kt = 2 * ktp + half
                            for j, rt in enumerate(grp):
                                nc.tensor.matmul(
                                    pst[rt][:],
                                    lhsT=xt[:, half, j * 128 : (j + 1) * 128],
                                    rhs=wv_t[kt][:],
                                    start=(kt == 0),
                                    stop=(kt == KT - 1),
                                )
                    for rt in grp:
                        if rt % 2 == 0:
                            nc.vector.tensor_copy(vv[:, rq * 8 + rt, :], pst[rt][:])
                        else:
                            nc.scalar.activation(vv[:, rq * 8 + rt, :], pst[rt][:], CopyF)

            # ---- attention per (b, local head) ----
            # st tiles span both 512-query blocks (2 PSUM banks) so Exp runs
            # once per key tile ([128,1024]: amortizes the ~352-cycle ACT
            # overhead). Emission is software-pipelined: st(mt+1) goes to the
            # PE queue before ot(mt), so the PE never waits on Exp latency.
            with (
                tc.tile_pool(name="stp", bufs=2, space="PSUM") as stp,
                tc.tile_pool(name="otp", bufs=1, space="PSUM") as otp,
                tc.tile_pool(name="ptp", bufs=6) as ptp,
                tc.tile_pool(name="osb", bufs=6) as osbp,
                tc.tile_pool(name="dtr", bufs=8) as dtrp,
            ):
                def emit_ot(b, hl, mt, ots, pts):
                    # lb-major for mt==0 and the last mt: their exps are split
                    # into lb halves, so both lb0 matmuls go first while the
                    # lb1 half finishes
                    pairs = (
                        [(vf, lb) for lb in range(N_LB) for vf in range(N_KF)]
                        if mt in (0, N_MT - 1)
                        else [(vf, lb) for vf in range(N_KF) for lb in range(N_LB)]
                    )
                    for vf, lb in pairs:
                        nc.tensor.matmul(
                            ots[vf * N_LB + lb][:],
                            lhsT=vv[
                                :,
                                b * 8 + mt,
                                hl * DH + vf * 128 : hl * DH + (vf + 1) * 128,
                            ],
                            rhs=pts[mt][:, lb * 512 : (lb + 1) * 512],
                            start=(mt == 0),
                            stop=(mt == N_MT - 1),
                        )

                def emit_den_final(b, hl, dacc, pt2):
                    # final den add + DMA, split into lb halves; emitted
                    # AFTER the ot evacs on the DVE queue (den gates nothing
                    # on the PE, the evacs gate the next iteration's ot(0));
                    # DMAs ride the otherwise-idle GpSimd queue
                    s = dtrp.tile([128, 1024], f32, tag="dtree", name="dts")
                    for half in range(2):
                        sl = slice(half * 512, (half + 1) * 512)
                        nc.vector.tensor_add(s[:, sl], dacc[:, sl], pt2[:, sl])
                        nc.gpsimd.dma_start(
                            out=den_d[hl * B + b, :, sl], in_=s[:, sl]
                        )

                def emit_evacs(b, hl, ots):
                    # bank order matches the lb-major stop order of the last
                    # ot pass, so DVE starts each copy as soon as its bank
                    # stops; split the store DMAs over two queues
                    for i in (0, 2, 1, 3):
                        vf, lb = divmod(i, N_LB)
                        ot_sb = osbp.tile([128, 512], bf16, tag="osb", name="ot_sb")
                        # evacuate on DVE only (ACT must stay exp-only, and
                        # GpSimd cannot read PSUM — fails walrus codegen)
                        nc.vector.tensor_copy(ot_sb, ots[i][:])
                        dma_eng = nc.sync if i % 2 == 0 else nc.scalar
                        dma_eng.dma_start(
                            out=outT_d[
                                hl * DH + vf * 128 : hl * DH + (vf + 1) * 128,
                                b * L + lb * 512 : b * L + (lb + 1) * 512,
                            ],
                            in_=ot_sb,
                        )

                # software-pipelined across (b, hl) iterations: the last ot
                # pass of iteration j is emitted AFTER iteration j+1's first
                # st group, so the PE chews on st(0') instead of stalling on
                # exp(7)'s latency at the boundary
                prev = None  # (b, hl, ots, pts) with ot(N_MT-1) still pending
                for b in range(B):
                    for hl in range(HPC):
                        ots = [
                            otp.tile([128, 512], f32, tag=f"ot{i}", name=f"ot{i}")
                            for i in range(N_KF * N_LB)
                        ]
                        pts = {}
                        dacc = None
                        for mt in range(N_MT):
                            st2 = stp.tile([128, 1024], f32, tag="st", name="st")
                            for lb in range(N_LB):
                                for kf in range(N_KF):
                                    nc.tensor.matmul(
                                        st2[:, lb * 512 : (lb + 1) * 512],
                                        lhsT=kT[HPC * hl + kf][
                                            :, b * L + mt * 128 : b * L + (mt + 1) * 128
                                        ],
                                        rhs=qT[HPC * hl + kf][
                                            :, b * L + lb * 512 : b * L + (lb + 1) * 512
                                        ],
                                        start=(kf == 0),
                                        stop=(kf == N_KF - 1),
                                    )
                            if mt == 0 and prev is not None:
                                pb, phl, pots, ppts, pdacc = prev
                                emit_ot(pb, phl, N_MT - 1, pots, ppts)
                                emit_evacs(pb, phl, pots)
                                emit_den_final(pb, phl, pdacc, ppts[N_MT - 1])
                                prev = None
                            pt2 = ptp.tile([128, 1024], bf16, tag="pt", name="pt")
                            if mt in (0, N_MT - 1):
                                # split the first and last Exp into halves: the
                                # first so ot(0) lands early, the last so the
                                # deferred ot(7) (emitted after the next
                                # iteration's st(0)) gets its lb0 half early.
                                nc.scalar.activation(
                                    pt2[:, 0:512], st2[:, 0:512], Exp
                                )
                                nc.scalar.activation(
                                    pt2[:, 512:1024], st2[:, 512:1024], Exp
                                )
                            else:
                                nc.scalar.activation(pt2[:], st2[:], Exp)
                            pts[mt] = pt2
                            # running denominator partial sum. Keep ALL of this
                            # on DVE: ACT must stay exp-only (evacs on its
                            # queue delay every next exp and the st->exp->ot
                            # chain cascades ~1.5us/iteration), and GpSimd is
                            # 2.1us per add (its chain becomes the kernel tail)
                            if mt == N_MT - 1:
                                pass  # final den add+DMA emitted in the flush
                            elif mt >= 1:
                                s = dtrp.tile([128, 1024], f32, tag="dtree", name="dts")
                                prev_p = pts[0] if mt == 1 else dacc
                                nc.vector.tensor_add(s[:], prev_p[:], pt2[:])
                                dacc = s
                            # defer ot by TWO key tiles so st(mt+1)+st(mt+2)
                            # cover exp(mt)'s latency (deeper deferral
                            # measured WORSE: 850ns vs 358ns boundary gaps)
                            if mt >= 2:
                                emit_ot(b, hl, mt - 2, ots, pts)
                        emit_ot(b, hl, N_MT - 2, ots, pts)
                        prev = (b, hl, ots, pts, dacc)
                pb, phl, pots, ppts, pdacc = prev
                emit_ot(pb, phl, N_MT - 1, pots, ppts)
                emit_evacs(pb, phl, pots)
                emit_den_final(pb, phl, pdacc, ppts[N_MT - 1])

    nc.compile()
    return nc


def _get_nc():
    if "nc" not in _CACHE:
        _CACHE["nc"] = _build_nc()
    return _CACHE["nc"]


def kernel(x, Wq, bq, Wk, bk, Wv, bv, bias_weight):
    from concourse.bass_utils import run_bass_kernel_spmd

    x = np.asarray(x, dtype=np.float32)
    Wq = np.asarray(Wq, dtype=np.float32)
    Wk = np.asarray(Wk, dtype=np.float32)
    Wv = np.asarray(Wv, dtype=np.float32)
    bq = np.asarray(bq, dtype=np.float32)
    bv = np.asarray(bv, dtype=np.float32)
    bias_weight = np.asarray(bias_weight, dtype=np.float32)

    # Fold the orientation bias + scale + blade reverse/metric into Wq.
    scale = 1.0 / np.sqrt(CD * NB)
    bivC = _C_NP[:, :, list(BIV_IDX)]  # [NB, NB, 3]
    wk_mix = np.einsum("ijc,hc->hij", bivC, bias_weight)  # [NH, NB, NB]
    A = _REV_NP[None, :, None] * wk_mix + scale * np.diag(_REV_NP * _MET_NP)[None]
    # Wq_eff[(h,d,j), f] = sum_i A[h,i,j] * Wq[(h,d,i), f]
    Wq4 = Wq.reshape(NH, CD, NB, DM)
    Wq_eff = np.matmul(Wq4.transpose(0, 1, 3, 2), A[:, None]).transpose(0, 1, 3, 2)
    Wq_eff = np.ascontiguousarray(Wq_eff).reshape(DM, DM)
    bq_eff = np.matmul(bq.reshape(NH, CD, NB)[:, :, None, :], A[:, None])
    bq_eff = bq_eff.reshape(DM).astype(np.float32)

    xT = np.ascontiguousarray(x.reshape(ROWS, DM).T).astype(BF16)

    nc = _get_nc()
    in_maps = []
    for c in range(N_CORES):
        sl = slice(c * FPC, (c + 1) * FPC)
        in_maps.append(
            {
                "xT": xT,
                "wqT": np.ascontiguousarray(Wq_eff[sl].T).astype(BF16),
                "wkT": np.ascontiguousarray(Wk[sl].T).astype(BF16),
                "wvT": np.ascontiguousarray(Wv[sl].T).astype(BF16),
                "bqe": np.ascontiguousarray(bq_eff[sl]),
            }
        )

    res = run_bass_kernel_spmd(
        nc,
        in_maps,
        core_ids=list(range(N_CORES)),
        trace=bool(int(os.environ.get("KERNEL_TRACE", "0"))),
    )
    _CACHE["last_results"] = res

    # Gather: out[b, l, c*FPC + hl*DH + f] = outT_c[hl*DH+f, b*L+l] / den_c[hl*B+b, l]
    parts = []
    for c in range(N_CORES):
        outT = np.asarray(res.results[c]["outT"], np.float32).reshape(HPC, DH, B, L)
        den = res.results[c]["den"].sum(axis=1).reshape(HPC, B, L)
        part = outT.transpose(2, 3, 0, 1) / den.transpose(1, 2, 0)[:, :, :, None]
        parts.append(part.reshape(B, L, FPC))
    out = np.concatenate(parts, axis=2)
    out += bv[None, None, :]
    return out.astype(np.float32)



# revision 24
# speedup vs baseline: 1.0084x; 1.0084x over previous
"""CliffordAttention TRN2 kernel.

Math: the reference's orientation-bias einsum chain is folded into the Wq
projection. With A_h[i,j] = scale*rev[j]*met[j]*delta_ij + rev[i]*wk[h,i,j]
(wk = biv_kernel . bias_weight), the total pre-softmax logits are exactly
q_eff . k where q_eff = x @ Wq_eff.T and Wq_eff mixes blades of Wq by A_h.
bk shifts every logit in a softmax row equally (drops out exactly); bv adds
bv to the output exactly (softmax rows sum to 1) and is applied on host.

Sharding: tensor-parallel over heads — core c computes heads 2c, 2c+1 for
all batches. x (feature-major, bf16) is replicated; W slices per core.
Device kernel per core: q/k projections in feature-major layout (qT/kT),
v in row-major, then per (b, head): S.T = k q^T (m on partitions), P=exp(S.T)
(no max subtraction needed: |logits| < ~20), out.T = v^T P accumulated on PE,
denominator partials as a DVE running sum (host reduces over partitions,
divides, transposes, concats).

The kernel is PE-bound at ~97% of the bf16 roofline (3512 N=512 matmuls,
~777us busy). Scheduling notes that keep it there: HAM warm-up matmuls
during the initial DMA wait; just-in-time weight-DMA spreading; kt-paired
3D x DMAs; in the attention phase, ACT runs exps only, DVE does denominator
adds + evacuations, ot matmuls trail their exp by two key tiles, and output
rides bf16 over two DMA queues. Measured HW exec ~802-808us/core; runs can
show +7-20% when the chip's P0 power throttle drops the PE to ~2GHz.
"""

import os

import numpy as np
import ml_dtypes

BF16 = ml_dtypes.bfloat16

# Problem shapes (hardcoded per contract).
B = 4
L = 1024
NH = 16
CD = 32
NB = 8
DM = NH * CD * NB  # 4096
DH = CD * NB  # 256 head dim
N_CORES = 8
HPC = NH // N_CORES  # heads per core
FPC = HPC * DH  # 512 output features per core
ROWS = B * L  # 4096
SIG_G = (1.0, 1.0, 1.0)
BIV_IDX = (4, 5, 6)


def _build_tables():
    """Geometric-product structure constants; copied from the model spec."""
    g = SIG_G
    dim = len(g)
    n = 2**dim
    masks = sorted(range(n), key=lambda m: (bin(m).count("1"), m))
    inv = {m: i for i, m in enumerate(masks)}
    C = np.zeros((n, n, n), dtype=np.float32)
    for i, a in enumerate(masks):
        for j, b in enumerate(masks):
            sw, at = 0, a >> 1
            while at:
                sw += bin(at & b).count("1")
                at >>= 1
            s = -1.0 if (sw & 1) else 1.0
            for t in range(dim):
                if (a >> t) & 1 and (b >> t) & 1:
                    s *= float(g[t])
            C[i, j, inv[a ^ b]] = s
    grades = np.array([bin(m).count("1") for m in masks])
    rev = np.where((grades * (grades - 1) // 2) % 2 == 1, -1.0, 1.0).astype(np.float32)
    met = np.array(
        [
            float(np.prod([g[t] for t in range(dim) if (m >> t) & 1] + [1.0]))
            for m in masks
        ],
        dtype=np.float32,
    )
    return C, rev, met


_C_NP, _REV_NP, _MET_NP = _build_tables()

# Device-kernel tiling constants.
KT = DM // 128  # 32 contraction tiles
NFT = FPC // 128  # 4 output-feature tiles per core
N_RQ = ROWS // 1024  # 4 row quads
N_MT = L // 128  # 8 key tiles per (b, h)
N_LB = L // 512  # 2 query blocks per (b, h)
N_KF = DH // 128  # 2 feature tiles per head

_CACHE = {}


def _build_nc():
    import concourse.tile as tile
    from concourse import bacc, mybir

    f32 = mybir.dt.float32
    bf16 = mybir.dt.bfloat16
    Exp = mybir.ActivationFunctionType.Exp
    CopyF = mybir.ActivationFunctionType.Copy
    IdentF = mybir.ActivationFunctionType.Identity

    nc = bacc.Bacc(
        "TRN2",
        target_bir_lowering=False,
        debug=False,
        enable_asserts=False,
        num_devices=N_CORES,
    )

    xT_d = nc.dram_tensor("xT", [DM, ROWS], bf16, kind="ExternalInput").ap()
    wqT_d = nc.dram_tensor("wqT", [DM, FPC], bf16, kind="ExternalInput").ap()
    wkT_d = nc.dram_tensor("wkT", [DM, FPC], bf16, kind="ExternalInput").ap()
    wvT_d = nc.dram_tensor("wvT", [DM, FPC], bf16, kind="ExternalInput").ap()
    bqe_d = nc.dram_tensor("bqe", [FPC], f32, kind="ExternalInput").ap()
    outT_d = nc.dram_tensor("outT", [FPC, ROWS], bf16, kind="ExternalOutput").ap()
    # denominator partial sums: summed over the partition axis on host
    den_d = nc.dram_tensor(
        "den", [HPC * B, 128, L], f32, kind="ExternalOutput"
    ).ap()

    with tile.TileContext(nc) as tc:
        # HAM warm-up: the PE clock-gate opens only after ~3.4us of sustained
        # matmul activity. The first real matmul can't start until weights+x
        # arrive (~10us); burn the DMA-wait window on throwaway matmuls so the
        # real stream starts at 2.4GHz instead of 1.2.
        with (
            tc.tile_pool(name="warm_ps", bufs=1, space="PSUM") as wpsp,
            tc.tile_pool(name="warm_sb", bufs=1) as wsbp,
        ):
            wsrc = wsbp.tile([128, 512], bf16, tag="wsrc", name="wsrc")
            nc.vector.memset(wsrc[:], 0)
            wps = wpsp.tile([128, 512], f32, tag="wps", name="wps")
            for _ in range(8):
                nc.tensor.matmul(
                    wps[:], lhsT=wsrc[:, 0:128], rhs=wsrc[:], start=True, stop=True
                )

        with tc.tile_pool(name="persist", bufs=1) as persist:
            qT = [
                persist.tile([128, ROWS], bf16, tag=f"qT{f}", name=f"qT{f}")
                for f in range(NFT)
            ]
            kT = [
                persist.tile([128, ROWS], bf16, tag=f"kT{f}", name=f"kT{f}")
                for f in range(NFT)
            ]
            vv = persist.tile([128, ROWS // 128, FPC], bf16, tag="vv", name="vv")
            bq_sb = persist.tile([128, NFT], f32, tag="bq", name="bq_sb")

            # ---- projections ----
            # Weights live in per-k-tile tiles (32 tags, bufs=2) so the first
            # matmul of a phase only waits on one small DMA, and the next
            # phase's weights prefetch during the current phase's compute.
            with (
                tc.tile_pool(name="wpool", bufs=2) as wp,
                tc.tile_pool(name="xs", bufs=16) as xp,
                tc.tile_pool(name="psp", bufs=1, space="PSUM") as pp,
            ):
                # Weight DMAs are spread just-in-time through the rblk loop:
                # issuing all 12.6MB upfront saturates HBM across the 8 cores
                # and starves the x stream (PE stalls ~45us in once the x
                # prefetch buffer drains).
                wq4 = wqT_d.rearrange("(kt p) f -> p kt f", p=128)
                wk4 = wkT_d.rearrange("(kt p) f -> p kt f", p=128)
                wv4 = wvT_d.rearrange("(kt p) f -> p kt f", p=128)
                x3 = xT_d.rearrange("(kt p) r -> p kt r", p=128)
                wq_t, wk_t, wv_t = [], [], []

                def issue_w(kind, kt):
                    lst, src = {"q": (wq_t, wq4), "k": (wk_t, wk4), "v": (wv_t, wv4)}[kind]
                    wt = wp.tile([128, FPC], bf16, tag=f"w{kt}", name=f"w{kind}{kt}")
                    nc.scalar.dma_start(out=wt, in_=src[:, kt, :])
                    lst.append(wt)

                # NOTE: splitting rblk0 into q-only + k-only passes (to halve
                # startup weight demand) was tried and is a NET LOSS (+17us):
                # the later passes' x DMAs get pulled forward by pool-buffer
                # rotation and starve whichever pass is running. Keep fused.
                QK_PREFETCH = 8
                for kt in range(QK_PREFETCH):
                    issue_w("q", kt)
                    issue_w("k", kt)
                nc.scalar.dma_start(
                    out=bq_sb, in_=bqe_d.rearrange("(f p) -> p f", p=128)
                )

                def proj_pass(rblk, do_q, do_k, w_sched, xpool=None):
                    """One 512-row pass; w_sched: ktp -> list of (kind, kt)."""
                    xpool = xpool or xp
                    pq = [
                        pp.tile([128, 512], f32, tag=f"ps{i}", name=f"psq{i}")
                        for i in range(NFT)
                    ] if do_q else None
                    pk = [
                        pp.tile([128, 512], f32, tag=f"ps{NFT + i}", name=f"psk{i}")
                        for i in range(NFT)
                    ] if do_k else None
                    # x rides in kt-PAIRED 3D DMAs ([128, 2, 512]): half the
                    # descriptor traffic on the Sync queue and twice the
                    # prefetch runway per pool buffer
                    for ktp in range(KT // 2):
                        for kind, kt in w_sched(ktp):
                            issue_w(kind, kt)
                        xt = xpool.tile([128, 2, 512], bf16, tag="x", name="xqkt")
                        nc.sync.dma_start(
                            out=xt,
                            in_=x3[
                                :,
                                2 * ktp : 2 * ktp + 2,
                                rblk * 512 : (rblk + 1) * 512,
                            ],
                        )
                        for half in range(2):
                            kt = 2 * ktp + half
                            if do_q:
                                for f in range(NFT):
                                    nc.tensor.matmul(
                                        pq[f][:],
                                        lhsT=wq_t[kt][:, f * 128 : (f + 1) * 128],
                                        rhs=xt[:, half, :],
                                        start=(kt == 0),
                                        stop=(kt == KT - 1),
                                    )
                            if do_k:
                                for f in range(NFT):
                                    nc.tensor.matmul(
                                        pk[f][:],
                                        lhsT=wk_t[kt][:, f * 128 : (f + 1) * 128],
                                        rhs=xt[:, half, :],
                                        start=(kt == 0),
                                        stop=(kt == KT - 1),
                                    )
                    # evacuate on both Vector and Scalar so the PSUM banks
                    # free ~2x faster at block boundaries
                    if do_q:
                        for f in range(NFT):
                            dst = qT[f][:, rblk * 512 : (rblk + 1) * 512]
                            if f % 2 == 0:
                                nc.vector.tensor_scalar_add(
                                    dst, pq[f][:], bq_sb[:, f : f + 1]
                                )
                            else:
                                nc.scalar.activation(
                                    dst, pq[f][:], IdentF, bias=bq_sb[:, f : f + 1]
                                )
                    if do_k:
                        # DVE takes k0,k2,k3; ACT only k1: ACT activations
                        # cost ~0.7us vs DVE's 0.43, and the latest-stopping
                        # bank (k3) on ACT misses its reuse window at the
                        # next rblk's group start (the steady 432ns PE gaps
                        # hit exactly the ACT-evacuated psum banks)
                        for f in range(NFT):
                            dst = kT[f][:, rblk * 512 : (rblk + 1) * 512]
                            if f == 1:
                                nc.scalar.activation(dst, pk[f][:], CopyF)
                            else:
                                nc.vector.tensor_copy(dst, pk[f][:])

                def rblk0_sched(ktp):
                    if ktp >= 12:
                        return []
                    out = []
                    for kt in (QK_PREFETCH + 2 * ktp, QK_PREFETCH + 2 * ktp + 1):
                        out += [("q", kt), ("k", kt)]
                    return out

                for rblk in range(ROWS // 512):
                    # weight DMAs spread just-in-time: rblk0 trickles the
                    # remaining wq/wk, rblk1 issues wv (gated behind rblk0
                    # evacs in the scalar queue so it can't starve x)
                    if rblk == 0:
                        sched = rblk0_sched
                    elif rblk == 1:
                        sched = lambda ktp: [("v", 2 * ktp), ("v", 2 * ktp + 1)]
                    else:
                        sched = lambda ktp: []
                    proj_pass(rblk, True, True, sched)

                # v projection: out[row_tile, f] = xT.T @ wvT (row-major v).
                # Uniform 4-bank half-quad passes: evacuations of one pass
                # overlap the next pass's matmuls, and at the v->attention
                # transition half the PSUM banks are already free (otherwise
                # the first st matmul waits ~1.2us on the final v evacs).
                # x DMAs are kt-paired like the q/k phase.
                for half_q in range(2 * N_RQ):
                    rq, pas = divmod(half_q, 2)
                    grp = [4 * pas + j for j in range(4)]
                    base = rq * 1024 + pas * 512
                    pst = {
                        rt: pp.tile([128, FPC], f32, tag=f"ps{rt}", name=f"psv{rt}")
                        for rt in grp
                    }
                    for ktp in range(KT // 2):
                        xt = xp.tile([128, 2, 512], bf16, tag="x", name="xvt")
                        nc.sync.dma_start(
                            out=xt,
                            in_=x3[:, 2 * ktp : 2 * ktp + 2, base : base + 512],
                        )
                        for half in range(2):
                            kt = 2 * ktp + half
                            for j, rt in enumerate(grp):
                                nc.tensor.matmul(
                                    pst[rt][:],
                                    lhsT=xt[:, half, j * 128 : (j + 1) * 128],
                                    rhs=wv_t[kt][:],
                                    start=(kt == 0),
                                    stop=(kt == KT - 1),
                                )
                    for rt in grp:
                        if rt % 2 == 0:
                            nc.vector.tensor_copy(vv[:, rq * 8 + rt, :], pst[rt][:])
                        else:
                            nc.scalar.activation(vv[:, rq * 8 + rt, :], pst[rt][:], CopyF)

            # ---- attention per (b, local head) ----
            # st tiles span both 512-query blocks (2 PSUM banks) so Exp runs
            # once per key tile ([128,1024]: amortizes the ~352-cycle ACT
            # overhead). Emission is software-pipelined: st(mt+1) goes to the
            # PE queue before ot(mt), so the PE never waits on Exp latency.
            with (
                tc.tile_pool(name="stp", bufs=2, space="PSUM") as stp,
                tc.tile_pool(name="otp", bufs=1, space="PSUM") as otp,
                tc.tile_pool(name="ptp", bufs=6) as ptp,
                tc.tile_pool(name="osb", bufs=6) as osbp,
                tc.tile_pool(name="dtr", bufs=8) as dtrp,
            ):
                def emit_ot(b, hl, mt, ots, pts):
                    # lb-major for mt==0 and the last mt: their exps are split
                    # into lb halves, so both lb0 matmuls go first while the
                    # lb1 half finishes
                    pairs = (
                        [(vf, lb) for lb in range(N_LB) for vf in range(N_KF)]
                        if mt in (0, N_MT - 1)
                        else [(vf, lb) for vf in range(N_KF) for lb in range(N_LB)]
                    )
                    for vf, lb in pairs:
                        nc.tensor.matmul(
                            ots[vf * N_LB + lb][:],
                            lhsT=vv[
                                :,
                                b * 8 + mt,
                                hl * DH + vf * 128 : hl * DH + (vf + 1) * 128,
                            ],
                            rhs=pts[mt][:, lb * 512 : (lb + 1) * 512],
                            start=(mt == 0),
                            stop=(mt == N_MT - 1),
                        )

                def emit_den_final(b, hl, dacc, pt2):
                    # final den add + DMA, split into lb halves; emitted
                    # AFTER the ot evacs on the DVE queue (den gates nothing
                    # on the PE, the evacs gate the next iteration's ot(0));
                    # DMAs ride the otherwise-idle GpSimd queue
                    s = dtrp.tile([128, 1024], f32, tag="dtree", name="dts")
                    for half in range(2):
                        sl = slice(half * 512, (half + 1) * 512)
                        nc.vector.tensor_add(s[:, sl], dacc[:, sl], pt2[:, sl])
                        nc.gpsimd.dma_start(
                            out=den_d[hl * B + b, :, sl], in_=s[:, sl]
                        )

                def emit_evacs(b, hl, ots):
                    # bank order matches the lb-major stop order of the last
                    # ot pass, so DVE starts each copy as soon as its bank
                    # stops; split the store DMAs over two queues
                    for i in (0, 2, 1, 3):
                        vf, lb = divmod(i, N_LB)
                        ot_sb = osbp.tile([128, 512], bf16, tag="osb", name="ot_sb")
                        # evacuate on DVE only (ACT must stay exp-only, and
                        # GpSimd cannot read PSUM — fails walrus codegen)
                        nc.vector.tensor_copy(ot_sb, ots[i][:])
                        dma_eng = nc.sync if i % 2 == 0 else nc.scalar
                        dma_eng.dma_start(
                            out=outT_d[
                                hl * DH + vf * 128 : hl * DH + (vf + 1) * 128,
                                b * L + lb * 512 : b * L + (lb + 1) * 512,
                            ],
                            in_=ot_sb,
                        )

                # software-pipelined across (b, hl) iterations: the last ot
                # pass of iteration j is emitted AFTER iteration j+1's first
                # st group, so the PE chews on st(0') instead of stalling on
                # exp(7)'s latency at the boundary
                prev = None  # (b, hl, ots, pts) with ot(N_MT-1) still pending
                for b in range(B):
                    for hl in range(HPC):
                        ots = [
                            otp.tile([128, 512], f32, tag=f"ot{i}", name=f"ot{i}")
                            for i in range(N_KF * N_LB)
                        ]
                        pts = {}
                        dacc = None
                        for mt in range(N_MT):
                            st2 = stp.tile([128, 1024], f32, tag="st", name="st")
                            for lb in range(N_LB):
                                for kf in range(N_KF):
                                    nc.tensor.matmul(
                                        st2[:, lb * 512 : (lb + 1) * 512],
                                        lhsT=kT[HPC * hl + kf][
                                            :, b * L + mt * 128 : b * L + (mt + 1) * 128
                                        ],
                                        rhs=qT[HPC * hl + kf][
                                            :, b * L + lb * 512 : b * L + (lb + 1) * 512
                                        ],
                                        start=(kf == 0),
                                        stop=(kf == N_KF - 1),
                                    )
                            if mt == 0 and prev is not None:
                                pb, phl, pots, ppts, pdacc = prev
                                emit_ot(pb, phl, N_MT - 1, pots, ppts)
                                emit_evacs(pb, phl, pots)
                                emit_den_final(pb, phl, pdacc, ppts[N_MT - 1])
                                prev = None
                            pt2 = ptp.tile([128, 1024], bf16, tag="pt", name="pt")
                            if mt in (0, N_MT - 1):
                                # split the first and last Exp into halves: the
                                # first so ot(0) lands early, the last so the
                                # deferred ot(7) (emitted after the next
                                # iteration's st(0)) gets its lb0 half early.
                                nc.scalar.activation(
                                    pt2[:, 0:512], st2[:, 0:512], Exp
                                )
                                nc.scalar.activation(
                                    pt2[:, 512:1024], st2[:, 512:1024], Exp
                                )
                            else:
                                nc.scalar.activation(pt2[:], st2[:], Exp)
                            pts[mt] = pt2
                            # running denominator partial sum. Keep ALL of this
                            # on DVE: ACT must stay exp-only (evacs on its
                            # queue delay every next exp and the st->exp->ot
                            # chain cascades ~1.5us/iteration), and GpSimd is
                            # 2.1us per add (its chain becomes the kernel tail)
                            if mt == N_MT - 1:
                                pass  # final den add+DMA emitted in the flush
                            elif mt >= 1:
                                s = dtrp.tile([128, 1024], f32, tag="dtree", name="dts")
                                prev_p = pts[0] if mt == 1 else dacc
                                nc.vector.tensor_add(s[:], prev_p[:], pt2[:])
                                dacc = s
                            # defer ot by TWO key tiles so st(mt+1)+st(mt+2)
                            # cover exp(mt)'s latency (deeper deferral
                            # measured WORSE: 850ns vs 358ns boundary gaps)
                            if mt >= 2:
                                emit_ot(b, hl, mt - 2, ots, pts)
                        emit_ot(b, hl, N_MT - 2, ots, pts)
                        prev = (b, hl, ots, pts, dacc)
                pb, phl, pots, ppts, pdacc = prev
                emit_ot(pb, phl, N_MT - 1, pots, ppts)
                emit_evacs(pb, phl, pots)
                emit_den_final(pb, phl, pdacc, ppts[N_MT - 1])

    nc.compile()
    return nc


def _get_nc():
    if "nc" not in _CACHE:
        _CACHE["nc"] = _build_nc()
    return _CACHE["nc"]


def kernel(x, Wq, bq, Wk, bk, Wv, bv, bias_weight):
    from concourse.bass_utils import run_bass_kernel_spmd

    x = np.asarray(x, dtype=np.float32)
    Wq = np.asarray(Wq, dtype=np.float32)
    Wk = np.asarray(Wk, dtype=np.float32)
    Wv = np.asarray(Wv, dtype=np.float32)
    bq = np.asarray(bq, dtype=np.float32)
    bv = np.asarray(bv, dtype=np.float32)
    bias_weight = np.asarray(bias_weight, dtype=np.float32)

    # Fold the orientation bias + scale + blade reverse/metric into Wq.
    scale = 1.0 / np.sqrt(CD * NB)
    bivC = _C_NP[:, :, list(BIV_IDX)]  # [NB, NB, 3]
    wk_mix = np.einsum("ijc,hc->hij", bivC, bias_weight)  # [NH, NB, NB]
    A = _REV_NP[None, :, None] * wk_mix + scale * np.diag(_REV_NP * _MET_NP)[None]
    # Wq_eff[(h,d,j), f] = sum_i A[h,i,j] * Wq[(h,d,i), f]
    Wq4 = Wq.reshape(NH, CD, NB, DM)
    Wq_eff = np.matmul(Wq4.transpose(0, 1, 3, 2), A[:, None]).transpose(0, 1, 3, 2)
    Wq_eff = np.ascontiguousarray(Wq_eff).reshape(DM, DM)
    bq_eff = np.matmul(bq.reshape(NH, CD, NB)[:, :, None, :], A[:, None])
    bq_eff = bq_eff.reshape(DM).astype(np.float32)

    xT = np.ascontiguousarray(x.reshape(ROWS, DM).T).astype(BF16)

    nc = _get_nc()
    in_maps = []
    for c in range(N_CORES):
        sl = slice(c * FPC, (c + 1) * FPC)
        in_maps.append(
            {
                "xT": xT,
                "wqT": np.ascontiguousarray(Wq_eff[sl].T).astype(BF16),
                "wkT": np.ascontiguousarray(Wk[sl].T).astype(BF16),
                "wvT": np.ascontiguousarray(Wv[sl].T).astype(BF16),
                "bqe": np.ascontiguousarray(bq_eff[sl]),
            }
        )

    res = run_bass_kernel_spmd(
        nc,
        in_maps,
        core_ids=list(range(N_CORES)),
        trace=bool(int(os.environ.get("KERNEL_TRACE", "0"))),
    )
    _CACHE["last_results"] = res

    # Gather: out[b, l, c*FPC + hl*DH + f] = outT_c[hl*DH+f, b*L+l] / den_c[hl*B+b, l]
    parts = []
    for c in range(N_CORES):
        outT = np.asarray(res.results[c]["outT"], np.float32).reshape(HPC, DH, B, L)
        den = res.results[c]["den"].sum(axis=1).reshape(HPC, B, L)
        part = outT.transpose(2, 3, 0, 1) / den.transpose(1, 2, 0)[:, :, :, None]
        parts.append(part.reshape(B, L, FPC))
    out = np.concatenate(parts, axis=2)
    out += bv[None, None, :]
    return out.astype(np.float32)

